# revision 31
# baseline (speedup 1.0000x reference)
"""BidafAttention Trainium2 kernel (fp8 DoubleRow, cross-batch pipelined).

score[b,l,r] = tanh( (lhs*w_prod) @ rhs^T + (lhs@w_l)[:,None] + (rhs@w_r)[None,:] + b )
a_lhs = softmax_R(score); a_rhs = softmax_L(score)
lhs_out = concat([lhs, a_lhs @ rhs], -1); rhs_out = concat([rhs, a_rhs^T @ lhs], -1)

Strategy: data-parallel over batch N=16 -> 2 batches per NeuronCore.
All three 1024^3 matmul groups (score, att_lhs, att_rhs) run in fp8e4
with perf_mode=DoubleRow (256-deep contraction per matmul via 3D
[128,2,F] APs); each stationary load streams both 512-wide output
chunks so the 256-column LDWEIGHTS hides behind the matmul pair.

lhsT carries w_prod and a x64 scale (raw lhs*w_prod values are e4m3
subnormals); tanh descales via scale=1/64 with u as per-partition bias;
64*v is added on DVE in the PSUM domain, staged through SBUF (a PSUM
RMW would be wiped by the next accumulation group's start-zeroing).
E = exp(tanh(S)) in fp8; rowsum rides exp's accum_out; colsum rides the
E^T PSUM-copies' accum_out after PE transposes (fp8 transpose outputs
land at element step 2 in PSUM).

Engine queues are in-order, so phases of the two batches are emitted
interleaved: score(b0) | score(b1) x att_rhs(b0) | att_lhs(b0) x
att_rhs(b1) | att_lhs(b1).  While ScalarE runs one batch's tanh/exp
chain, the PE streams the other batch's attention matmuls, keeping the
HAM clock gate at full speed.  Outputs are written bf16, upcast on host.
"""

import sys

for _p in ("/opt/trn_rl_repo",):
    if _p not in sys.path:
        sys.path.insert(0, _p)

import numpy as np
import ml_dtypes

import concourse.tile as tile
import concourse.mybir as mybir
from concourse import bacc
from concourse.bass_utils import run_bass_kernel_spmd

AF = mybir.ActivationFunctionType
DR = mybir.MatmulPerfMode.DoubleRow
BF16 = mybir.dt.bfloat16
F8 = mybir.dt.float8e4
F32 = mybir.dt.float32

P = 128
SEQ = 1024  # L == R == D == 1024
NT = SEQ // P  # 8 tiles per dim
NK2 = NT // 2  # 4 double-row contraction steps
CH = 512  # psum chunk (free dim)
NCH = SEQ // CH  # 2
NB = 2  # batches per core
N_CORES = 8
D = 1024
W_SCALE = 64.0  # folded into lhs_t on host; descaled in the tanh activation
N_WARMUP = 60   # dummy PE ops at start to lift the HAM clock gate; sized
                # to end right as the score operands' DMA completes

_nc_cache = None


def _build_program():
    nc = bacc.Bacc("TRN2", target_bir_lowering=False, debug=False, num_devices=N_CORES)

    # fp8 operands, host-swizzled so each SBUF tile is one contiguous
    # [P, NT*SEQ] block: arr[b, p, k, :] = src[b, k*128+p, :]
    lhs_n = nc.declare_dram_parameter("lhs_n", [NB, P, NT, D], F8, isOutput=False)
    rhs_n = nc.declare_dram_parameter("rhs_n", [NB, P, NT, D], F8, isOutput=False)
    lhs_t = nc.declare_dram_parameter("lhs_t", [NB, P, NT, SEQ], F8, isOutput=False)
    rhs_t = nc.declare_dram_parameter("rhs_t", [NB, P, NT, SEQ], F8, isOutput=False)
    u_d = nc.declare_dram_parameter("u", [NB, P, NT], F32, isOutput=False)
    vb_d = nc.declare_dram_parameter("vb", [NB, P, SEQ], BF16, isOutput=False)
    id_d = nc.declare_dram_parameter("id_f8", [P, P], F8, isOutput=False)
    att_lhs = nc.declare_dram_parameter("att_lhs", [NB, SEQ, D], BF16, isOutput=True)
    att_rhs = nc.declare_dram_parameter("att_rhs", [NB, SEQ, D], BF16, isOutput=True)

    from contextlib import ExitStack

    with tile.TileContext(nc) as tc, ExitStack() as ctx:
        const = ctx.enter_context(tc.tile_pool(name="const", bufs=1))
        ident = const.tile([P, P], F8)
        nc.sync.dma_start(ident[:], id_d[:])

        pool_in = ctx.enter_context(tc.tile_pool(name="inf8", bufs=2))
        pool_tr = ctx.enter_context(tc.tile_pool(name="trf8", bufs=2))
        pool_e = ctx.enter_context(tc.tile_pool(name="ef8", bufs=2))
        pool_T = ctx.enter_context(tc.tile_pool(name="tanh", bufs=3))
        pool_sm = ctx.enter_context(tc.tile_pool(name="small", bufs=2))
        pool_out = ctx.enter_context(tc.tile_pool(name="osb", bufs=4))
        pool_dram = ctx.enter_context(tc.tile_pool(name="scr", bufs=1, space="DRAM"))
        psum_s = ctx.enter_context(tc.tile_pool(name="ps_s", bufs=2, space="PSUM"))
        psum_tr = ctx.enter_context(tc.tile_pool(name="ps_tr", bufs=2, space="PSUM"))
        psum_o = ctx.enter_context(tc.tile_pool(name="ps_o", bufs=2, space="PSUM"))

        # --- PE warmup: keep TensorE busy from t=0 so the HAM clock gate
        # opens before the first real matmul arrives. fp8 transpose-mode
        # output must land with element step 2 in PSUM.
        wps = psum_tr.tile([P, CH, 2], F8, tag="ptr", name="warm_ps")
        for _ in range(N_WARMUP):
            nc.tensor.transpose(wps[:, 0:P, 0], ident[:], ident[:])
        wsb = const.tile([P, P], F8, name="warm_sb")
        nc.scalar.copy(wsb[:], wps[:, 0:P, 0])

        # ---- per-batch state ----
        st = [dict() for _ in range(NB)]

        def load_batch(b):
            s = st[b]
            s["u"] = pool_sm.tile([P, NT], F32, tag="u", name=f"u{b}")
            nc.sync.dma_start(s["u"][:], u_d[b])
            s["vb"] = pool_sm.tile([P, SEQ], BF16, tag="vb", name=f"vb{b}")
            nc.sync.dma_start(s["vb"][:], vb_d[b])
            # score operands first (rhsT fully, then lhsT), then naturals
            for nm, dram in (("rhsT", rhs_t), ("lhsT", lhs_t),
                             ("lhs_n", lhs_n), ("rhs_n", rhs_n)):
                pool = pool_tr if nm in ("rhsT", "lhsT") else pool_in
                t = pool.tile([P, NT, SEQ], F8, tag=nm, name=f"{nm}{b}")
                s[nm] = t
                # one descriptor per tensor: 8KB contiguous per partition
                # maximizes DMA packet size (single queue, 16 engines)
                nc.sync.dma_start(t[:, :, :], dram[b])
            s["E"] = pool_e.tile([P, NT, SEQ], F8, tag="E", name=f"E{b}")
            s["E_T"] = pool_e.tile([P, NT, SEQ], F8, tag="E_T", name=f"E_T{b}")
            s["rowsum"] = pool_sm.tile([P, NT], F32, tag="rowsum", name=f"rowsum{b}")
            s["cparts"] = pool_sm.tile([P, 2, NT], F32, tag="cparts", name=f"cparts{b}")
            s["r_row"] = pool_sm.tile([P, NT], F32, tag="rrow", name=f"rrow{b}")
            s["r_col"] = pool_sm.tile([P, NT], F32, tag="rcol", name=f"rcol{b}")

        def score_step(b, i):
            """S[i] = (lhsT)^T @ rhsT (DoubleRow) -> +v (DVE) -> tanh -> exp/rowsum."""
            s = st[b]
            S = psum_s.tile([P, NCH, CH], F32, tag="ps", name=f"S{b}_{i}")
            for k2 in range(NK2):
                w_ap = s["lhsT"][:, 2 * k2:2 * k2 + 2, i * P:(i + 1) * P]
                for jc in range(NCH):
                    nc.tensor.matmul(
                        S[:, jc, :],
                        w_ap,
                        s["rhsT"][:, 2 * k2:2 * k2 + 2, jc * CH:(jc + 1) * CH],
                        start=(k2 == 0),
                        stop=(k2 == NK2 - 1),
                        perf_mode=DR,
                    )
            Sv = pool_T.tile([P, SEQ], F32, tag="Sv", name=f"Sv{b}_{i}")
            # Sv = S + 64*v on DVE, PSUM -> SBUF (PSUM RMW would be wiped
            # by the next group's start-zeroing)
            S_flat = S[:].rearrange("p a b -> p (a b)")
            nc.vector.tensor_add(Sv[:], S_flat, s["vb"][:])
            T_t = pool_T.tile([P, SEQ], F32, tag="T", name=f"T{b}_{i}")
            # T = tanh(Sv/64 + u[l]); u enters as the per-partition bias
            nc.scalar.activation(
                T_t[:], Sv[:], AF.Tanh,
                bias=s["u"][:, i:i + 1], scale=1.0 / W_SCALE,
            )
            # E = exp(T) (fp8); rowsum via DVE reduce
            nc.scalar.activation(s["E"][:, i, :], T_t[:], AF.Exp)
            nc.vector.reduce_sum(
                s["rowsum"][:, i:i + 1], s["E"][:, i, :], axis=mybir.AxisListType.X
            )
            if i == NT - 1:
                nc.vector.reciprocal(s["r_row"][:], s["rowsum"][:])

        def att_rhs_step(b, j):
            """transposes -> E_T chunk copies (+colsum) and
            att_rhs[r,d] = (1/colsum[r]) * sum_l E[l,r] lhs[l,d]."""
            s = st[b]
            po_tiles = []
            for half in range(2):
                pt = psum_tr.tile([P, CH, 2], F8, tag="ptr", name=f"pte{b}_{j}_{half}")
                for q in range(4):
                    i = half * 4 + q
                    nc.tensor.transpose(
                        pt[:, q * P:(q + 1) * P, 0],
                        s["E"][:, i, j * P:(j + 1) * P],
                        ident[:],
                    )
                # copy + partial colsum (sum over this 512-wide l-chunk)
                nc.scalar.activation(
                    s["E_T"][:, j, half * CH:(half + 1) * CH],
                    pt[:, :, 0],
                    AF.Copy,
                    accum_out=s["cparts"][:, half, j:j + 1],
                )
                dc = half
                po = psum_o.tile([P, CH], F32, tag="po", name=f"por{b}_{j}_{dc}")
                for k2 in range(NK2):
                    nc.tensor.matmul(
                        po[:],
                        s["E"][:, 2 * k2:2 * k2 + 2, j * P:(j + 1) * P],
                        s["lhs_n"][:, 2 * k2:2 * k2 + 2, dc * CH:(dc + 1) * CH],
                        start=(k2 == 0),
                        stop=(k2 == NK2 - 1),
                        perf_mode=DR,
                    )
                po_tiles.append(po)
            nc.vector.tensor_add(
                s["r_col"][:, j:j + 1],
                s["cparts"][:, 0, j:j + 1], s["cparts"][:, 1, j:j + 1],
            )
            nc.vector.reciprocal(s["r_col"][:, j:j + 1], s["r_col"][:, j:j + 1])
            osb = pool_out.tile([P, SEQ], BF16, tag="osb", name=f"or{b}_{j}")
            for dc in range(NCH):
                nc.vector.tensor_scalar_mul(
                    osb[:, dc * CH:(dc + 1) * CH], po_tiles[dc][:], s["r_col"][:, j:j + 1]
                )
            nc.sync.dma_start(att_rhs[b, j * P:(j + 1) * P, :], osb[:])

        def att_lhs_step(b, i):
            """att_lhs[l,d] = (1/rowsum[l]) * sum_r E[l,r] rhs[r,d]."""
            s = st[b]
            osb = pool_out.tile([P, SEQ], BF16, tag="osb", name=f"ol{b}_{i}")
            for dc in range(NCH):
                po = psum_o.tile([P, CH], F32, tag="po", name=f"pol{b}_{i}_{dc}")
                for k2 in range(NK2):
                    nc.tensor.matmul(
                        po[:],
                        s["E_T"][:, 2 * k2:2 * k2 + 2, i * P:(i + 1) * P],
                        s["rhs_n"][:, 2 * k2:2 * k2 + 2, dc * CH:(dc + 1) * CH],
                        start=(k2 == 0),
                        stop=(k2 == NK2 - 1),
                        perf_mode=DR,
                    )
                nc.vector.tensor_scalar_mul(
                    osb[:, dc * CH:(dc + 1) * CH], po[:], s["r_row"][:, i:i + 1]
                )
            nc.sync.dma_start(att_lhs[b, i * P:(i + 1) * P, :], osb[:])

        # ---- pipelined emission ----
        load_batch(0)
        load_batch(1)
        # warmup sink: a DRAM write keeps the warmup chain live; emitted
        # here so it drains during the prologue instead of the tail
        warm_dram = pool_dram.tile([P, P], F8, tag="warm", name="warm_dram")
        nc.sync.dma_start(warm_dram[:], wsb[:])
        for i in range(NT):
            score_step(0, i)
        for step in range(NT):           # score(b1) x att_rhs(b0)
            score_step(1, step)
            att_rhs_step(0, step)
        for step in range(NT):           # att_lhs(b0) x att_rhs(b1)
            att_lhs_step(0, step)
            att_rhs_step(1, step)
        for i in range(NT):              # tail
            att_lhs_step(1, i)

    nc.compile()
    return nc


def _get_nc():
    global _nc_cache
    if _nc_cache is None:
        _nc_cache = _build_program()
    return _nc_cache


def _prepare_in_maps(lhs, rhs, w, b):
    lhs = np.ascontiguousarray(lhs, dtype=np.float32)
    rhs = np.ascontiguousarray(rhs, dtype=np.float32)
    w = np.asarray(w, dtype=np.float32)
    b = np.float32(b)
    w_prod, w_l, w_r = w[:D], w[D:2 * D], w[2 * D:]

    # tiny host matvecs (exact, fp32)
    u_full = lhs @ w_l + b  # (N, L)
    v_full = rhs @ w_r      # (N, R)

    f8 = ml_dtypes.float8_e4m3
    bf = ml_dtypes.bfloat16
    id_f8 = np.eye(P, dtype=f8)
    N = lhs.shape[0]

    def swizzle(a):
        # (N, X, Y) -> (N, P, NT, Y) with out[n, p, k] = a[n, k*128+p]
        return np.ascontiguousarray(
            a.reshape(N, NT, P, a.shape[2]).transpose(0, 2, 1, 3)
        )

    lhs_n = swizzle(lhs.astype(f8).astype(np.float32)).astype(f8)
    rhs_n = swizzle(rhs.astype(f8).astype(np.float32)).astype(f8)
    # d-major score operands; w_prod (and the fp8 range scale) fold into lhs^T
    lhs_tm = np.ascontiguousarray((lhs * (w_prod * W_SCALE)).transpose(0, 2, 1))
    rhs_tm = np.ascontiguousarray(rhs.transpose(0, 2, 1))
    lhs_t = swizzle(lhs_tm).astype(f8)
    rhs_t = swizzle(rhs_tm).astype(f8)

    in_maps = []
    for c in range(N_CORES):
        b0 = c * NB
        u_arr = np.ascontiguousarray(
            u_full[b0:b0 + NB].reshape(NB, NT, P).transpose(0, 2, 1)
        )  # (NB, 128, 8)
        # v is added on DVE in the PSUM (x64) domain, pre-descale
        v_bf = (v_full[b0:b0 + NB] * W_SCALE).astype(bf)  # (NB, R)
        vb_arr = np.ascontiguousarray(
            np.broadcast_to(v_bf[:, None, :], (NB, P, SEQ))
        )
        in_maps.append(
            {
                "lhs_n": lhs_n[b0:b0 + NB],
                "rhs_n": rhs_n[b0:b0 + NB],
                "lhs_t": lhs_t[b0:b0 + NB],
                "rhs_t": rhs_t[b0:b0 + NB],
                "u": u_arr,
                "vb": vb_arr,
                "id_f8": id_f8,
            }
        )
    return in_maps


def run_device(lhs, rhs, w, b, trace=False):
    """Returns (att_lhs, att_rhs, BassKernelResults)."""
    nc = _get_nc()
    in_maps = _prepare_in_maps(lhs, rhs, w, b)
    res = run_bass_kernel_spmd(
        nc, in_maps, core_ids=list(range(N_CORES)), trace=trace
    )
    N = lhs.shape[0]
    bf = ml_dtypes.bfloat16
    att_lhs = np.empty((N, SEQ, D), dtype=np.float32)
    att_rhs = np.empty((N, SEQ, D), dtype=np.float32)
    for c in range(N_CORES):
        b0 = c * NB
        att_lhs[b0:b0 + NB] = np.asarray(res.results[c]["att_lhs"]).view(bf).astype(np.float32)
        att_rhs[b0:b0 + NB] = np.asarray(res.results[c]["att_rhs"]).view(bf).astype(np.float32)
    return att_lhs, att_rhs, res


def kernel(lhs, rhs, w, b):
    import os

    lhs = np.asarray(lhs, dtype=np.float32)
    rhs = np.asarray(rhs, dtype=np.float32)
    assert lhs.shape == (N_CORES * NB, SEQ, D) and rhs.shape == lhs.shape, (
        f"expected ({N_CORES * NB}, {SEQ}, {D}) inputs, got {lhs.shape}/{rhs.shape}"
    )
    had = os.environ.get("BASS_NEVER_TRACE")
    os.environ["BASS_NEVER_TRACE"] = "1"
    try:
        att_lhs, att_rhs, _ = run_device(lhs, rhs, w, b, trace=False)
    finally:
        if had is None:
            os.environ.pop("BASS_NEVER_TRACE", None)
        else:
            os.environ["BASS_NEVER_TRACE"] = had
    lhs_out = np.concatenate([lhs, att_lhs], axis=2)
    rhs_out = np.concatenate([rhs, att_rhs], axis=2)
    return lhs_out, rhs_out


# revision 32
# speedup vs baseline: 1.2165x; 1.2165x over previous
"""BidafAttention Trainium2 kernel (fp8 DoubleRow, cross-batch pipelined).

score[b,l,r] = tanh( (lhs*w_prod) @ rhs^T + (lhs@w_l)[:,None] + (rhs@w_r)[None,:] + b )
a_lhs = softmax_R(score); a_rhs = softmax_L(score)
lhs_out = concat([lhs, a_lhs @ rhs], -1); rhs_out = concat([rhs, a_rhs^T @ lhs], -1)

Strategy: data-parallel over batch N=16 -> 2 batches per NeuronCore.
All three 1024^3 matmul groups (score, att_lhs, att_rhs) run in fp8e4
with perf_mode=DoubleRow (256-deep contraction per matmul via 3D
[128,2,F] APs); each stationary load streams both 512-wide output
chunks so the 256-column LDWEIGHTS hides behind the matmul pair.

lhsT carries w_prod and a x64 scale (raw lhs*w_prod values are e4m3
subnormals); tanh descales via scale=1/64 with u as per-partition bias;
64*v is added on DVE in the PSUM domain, staged through SBUF (a PSUM
RMW would be wiped by the next accumulation group's start-zeroing).
E = exp(tanh(S)) in fp8; rowsum rides exp's accum_out; colsum rides the
E^T PSUM-copies' accum_out after PE transposes (fp8 transpose outputs
land at element step 2 in PSUM).

Engine queues are in-order, so phases of the two batches are emitted
interleaved: score(b0) | score(b1) x att_rhs(b0) | att_lhs(b0) x
att_rhs(b1) | att_lhs(b1).  While ScalarE runs one batch's tanh/exp
chain, the PE streams the other batch's attention matmuls, keeping the
HAM clock gate at full speed.  Outputs are written bf16, upcast on host.
"""

import sys

for _p in ("/opt/trn_rl_repo",):
    if _p not in sys.path:
        sys.path.insert(0, _p)

import numpy as np
import ml_dtypes

import concourse.tile as tile
import concourse.mybir as mybir
from concourse import bacc
from concourse.bass_utils import run_bass_kernel_spmd

AF = mybir.ActivationFunctionType
DR = mybir.MatmulPerfMode.DoubleRow
BF16 = mybir.dt.bfloat16
F8 = mybir.dt.float8e4
F32 = mybir.dt.float32

P = 128
SEQ = 1024  # L == R == D == 1024
NT = SEQ // P  # 8 tiles per dim
NK2 = NT // 2  # 4 double-row contraction steps
CH = 512  # psum chunk (free dim)
NCH = SEQ // CH  # 2
NB = 2  # batches per core
N_CORES = 8
D = 1024
W_SCALE = 64.0  # folded into lhs_t on host; descaled in the tanh activation
N_WARMUP = 100  # dummy PE ops at start to lift the HAM clock gate; sized
                # to end right as the score operands' DMA completes

_nc_cache = None


def _build_program():
    nc = bacc.Bacc("TRN2", target_bir_lowering=False, debug=False, num_devices=N_CORES)

    # fp8 operands, host-swizzled so each SBUF tile is one contiguous
    # [P, NT*SEQ] block: arr[b, p, k, :] = src[b, k*128+p, :]
    lhs_n = nc.declare_dram_parameter("lhs_n", [NB, P, NT, D], F8, isOutput=False)
    rhs_n = nc.declare_dram_parameter("rhs_n", [NB, P, NT, D], F8, isOutput=False)
    lhs_t = nc.declare_dram_parameter("lhs_t", [NB, P, NT, SEQ], F8, isOutput=False)
    rhs_t = nc.declare_dram_parameter("rhs_t", [NB, P, NT, SEQ], F8, isOutput=False)
    u_d = nc.declare_dram_parameter("u", [NB, P, NT], F32, isOutput=False)
    vb_d = nc.declare_dram_parameter("vb", [NB, P, SEQ], BF16, isOutput=False)
    id_d = nc.declare_dram_parameter("id_f8", [P, P], F8, isOutput=False)
    att_lhs = nc.declare_dram_parameter("att_lhs", [NB, SEQ, D], BF16, isOutput=True)
    att_rhs = nc.declare_dram_parameter("att_rhs", [NB, SEQ, D], BF16, isOutput=True)

    from contextlib import ExitStack

    with tile.TileContext(nc) as tc, ExitStack() as ctx:
        const = ctx.enter_context(tc.tile_pool(name="const", bufs=1))
        ident = const.tile([P, P], F8)
        nc.sync.dma_start(ident[:], id_d[:])

        pool_in = ctx.enter_context(tc.tile_pool(name="inf8", bufs=2))
        pool_tr = ctx.enter_context(tc.tile_pool(name="trf8", bufs=2))
        pool_e = ctx.enter_context(tc.tile_pool(name="ef8", bufs=2))
        pool_T = ctx.enter_context(tc.tile_pool(name="tanh", bufs=3))
        pool_sm = ctx.enter_context(tc.tile_pool(name="small", bufs=2))
        pool_out = ctx.enter_context(tc.tile_pool(name="osb", bufs=4))
        pool_dram = ctx.enter_context(tc.tile_pool(name="scr", bufs=1, space="DRAM"))
        psum_s = ctx.enter_context(tc.tile_pool(name="ps_s", bufs=2, space="PSUM"))
        psum_tr = ctx.enter_context(tc.tile_pool(name="ps_tr", bufs=2, space="PSUM"))
        psum_o = ctx.enter_context(tc.tile_pool(name="ps_o", bufs=2, space="PSUM"))

        # --- PE warmup: keep TensorE busy from t=0 so the HAM clock gate
        # opens before the first real matmul arrives. fp8 transpose-mode
        # output must land with element step 2 in PSUM.
        wps = psum_tr.tile([P, CH, 2], F8, tag="ptr", name="warm_ps")
        for _ in range(N_WARMUP):
            nc.tensor.transpose(wps[:, 0:P, 0], ident[:], ident[:])
        wsb = const.tile([P, P], F8, name="warm_sb")
        nc.scalar.copy(wsb[:], wps[:, 0:P, 0])

        # ---- per-batch state ----
        st = [dict() for _ in range(NB)]

        def load_batch(b):
            s = st[b]
            s["u"] = pool_sm.tile([P, NT], F32, tag="u", name=f"u{b}")
            nc.sync.dma_start(s["u"][:], u_d[b])
            s["vb"] = pool_sm.tile([P, SEQ], BF16, tag="vb", name=f"vb{b}")
            nc.sync.dma_start(s["vb"][:], vb_d[b])
            # score operands first (rhsT fully, then lhsT), then naturals
            for nm, dram in (("rhsT", rhs_t), ("lhsT", lhs_t),
                             ("lhs_n", lhs_n), ("rhs_n", rhs_n)):
                pool = pool_tr if nm in ("rhsT", "lhsT") else pool_in
                t = pool.tile([P, NT, SEQ], F8, tag=nm, name=f"{nm}{b}")
                s[nm] = t
                # one descriptor per tensor: 8KB contiguous per partition
                # maximizes DMA packet size (single queue, 16 engines)
                nc.sync.dma_start(t[:, :, :], dram[b])
            s["E"] = pool_e.tile([P, NT, SEQ], F8, tag="E", name=f"E{b}")
            s["E_T"] = pool_e.tile([P, NT, SEQ], F8, tag="E_T", name=f"E_T{b}")
            s["rowsum"] = pool_sm.tile([P, NT], F32, tag="rowsum", name=f"rowsum{b}")
            s["cparts"] = pool_sm.tile([P, 2, NT], F32, tag="cparts", name=f"cparts{b}")
            s["r_row"] = pool_sm.tile([P, NT], F32, tag="rrow", name=f"rrow{b}")
            s["r_col"] = pool_sm.tile([P, NT], F32, tag="rcol", name=f"rcol{b}")

        def score_step(b, i):
            """S[i] = (lhsT)^T @ rhsT (DoubleRow) -> +v (DVE) -> tanh -> exp/rowsum."""
            s = st[b]
            S = psum_s.tile([P, NCH, CH], F32, tag="ps", name=f"S{b}_{i}")
            for k2 in range(NK2):
                w_ap = s["lhsT"][:, 2 * k2:2 * k2 + 2, i * P:(i + 1) * P]
                for jc in range(NCH):
                    nc.tensor.matmul(
                        S[:, jc, :],
                        w_ap,
                        s["rhsT"][:, 2 * k2:2 * k2 + 2, jc * CH:(jc + 1) * CH],
                        start=(k2 == 0),
                        stop=(k2 == NK2 - 1),
                        perf_mode=DR,
                    )
            Sv = pool_T.tile([P, SEQ], F32, tag="Sv", name=f"Sv{b}_{i}")
            # Sv = S + 64*v on DVE, PSUM -> SBUF (PSUM RMW would be wiped
            # by the next group's start-zeroing)
            S_flat = S[:].rearrange("p a b -> p (a b)")
            nc.vector.tensor_add(Sv[:], S_flat, s["vb"][:])
            T_t = pool_T.tile([P, SEQ], F32, tag="T", name=f"T{b}_{i}")
            # T = tanh(Sv/64 + u[l]); u enters as the per-partition bias
            nc.scalar.activation(
                T_t[:], Sv[:], AF.Tanh,
                bias=s["u"][:, i:i + 1], scale=1.0 / W_SCALE,
            )
            # E = exp(T) (fp8); rowsum via DVE reduce
            nc.scalar.activation(s["E"][:, i, :], T_t[:], AF.Exp)
            nc.vector.reduce_sum(
                s["rowsum"][:, i:i + 1], s["E"][:, i, :], axis=mybir.AxisListType.X
            )
            if i == NT - 1:
                nc.vector.reciprocal(s["r_row"][:], s["rowsum"][:])

        def att_rhs_step(b, j):
            """transposes -> E_T chunk copies (+colsum) and
            att_rhs[r,d] = (1/colsum[r]) * sum_l E[l,r] lhs[l,d]."""
            s = st[b]
            po_tiles = []
            for half in range(2):
                pt = psum_tr.tile([P, CH, 2], F8, tag="ptr", name=f"pte{b}_{j}_{half}")
                for q in range(4):
                    i = half * 4 + q
                    nc.tensor.transpose(
                        pt[:, q * P:(q + 1) * P, 0],
                        s["E"][:, i, j * P:(j + 1) * P],
                        ident[:],
                    )
                # copy + partial colsum (sum over this 512-wide l-chunk)
                nc.scalar.activation(
                    s["E_T"][:, j, half * CH:(half + 1) * CH],
                    pt[:, :, 0],
                    AF.Copy,
                    accum_out=s["cparts"][:, half, j:j + 1],
                )
                dc = half
                po = psum_o.tile([P, CH], F32, tag="po", name=f"por{b}_{j}_{dc}")
                for k2 in range(NK2):
                    nc.tensor.matmul(
                        po[:],
                        s["E"][:, 2 * k2:2 * k2 + 2, j * P:(j + 1) * P],
                        s["lhs_n"][:, 2 * k2:2 * k2 + 2, dc * CH:(dc + 1) * CH],
                        start=(k2 == 0),
                        stop=(k2 == NK2 - 1),
                        perf_mode=DR,
                    )
                po_tiles.append(po)
            nc.vector.tensor_add(
                s["r_col"][:, j:j + 1],
                s["cparts"][:, 0, j:j + 1], s["cparts"][:, 1, j:j + 1],
            )
            nc.vector.reciprocal(s["r_col"][:, j:j + 1], s["r_col"][:, j:j + 1])
            osb = pool_out.tile([P, SEQ], BF16, tag="osb", name=f"or{b}_{j}")
            for dc in range(NCH):
                nc.vector.tensor_scalar_mul(
                    osb[:, dc * CH:(dc + 1) * CH], po_tiles[dc][:], s["r_col"][:, j:j + 1]
                )
            nc.sync.dma_start(att_rhs[b, j * P:(j + 1) * P, :], osb[:])

        def att_lhs_step(b, i):
            """att_lhs[l,d] = (1/rowsum[l]) * sum_r E[l,r] rhs[r,d]."""
            s = st[b]
            osb = pool_out.tile([P, SEQ], BF16, tag="osb", name=f"ol{b}_{i}")
            for dc in range(NCH):
                po = psum_o.tile([P, CH], F32, tag="po", name=f"pol{b}_{i}_{dc}")
                for k2 in range(NK2):
                    nc.tensor.matmul(
                        po[:],
                        s["E_T"][:, 2 * k2:2 * k2 + 2, i * P:(i + 1) * P],
                        s["rhs_n"][:, 2 * k2:2 * k2 + 2, dc * CH:(dc + 1) * CH],
                        start=(k2 == 0),
                        stop=(k2 == NK2 - 1),
                        perf_mode=DR,
                    )
                nc.vector.tensor_scalar_mul(
                    osb[:, dc * CH:(dc + 1) * CH], po[:], s["r_row"][:, i:i + 1]
                )
            nc.sync.dma_start(att_lhs[b, i * P:(i + 1) * P, :], osb[:])

        # ---- pipelined emission ----
        load_batch(0)
        load_batch(1)
        # warmup sink: a DRAM write keeps the warmup chain live; emitted
        # here so it drains during the prologue instead of the tail
        warm_dram = pool_dram.tile([P, P], F8, tag="warm", name="warm_dram")
        nc.sync.dma_start(warm_dram[:], wsb[:])
        for i in range(NT):
            score_step(0, i)
        for step in range(NT):           # score(b1) x att_rhs(b0)
            score_step(1, step)
            att_rhs_step(0, step)
        for step in range(NT):           # att_lhs(b0) x att_rhs(b1)
            att_lhs_step(0, step)
            att_rhs_step(1, step)
        for i in range(NT):              # tail
            att_lhs_step(1, i)

    nc.compile()
    return nc


def _get_nc():
    global _nc_cache
    if _nc_cache is None:
        _nc_cache = _build_program()
    return _nc_cache


def _prepare_in_maps(lhs, rhs, w, b):
    lhs = np.ascontiguousarray(lhs, dtype=np.float32)
    rhs = np.ascontiguousarray(rhs, dtype=np.float32)
    w = np.asarray(w, dtype=np.float32)
    b = np.float32(b)
    w_prod, w_l, w_r = w[:D], w[D:2 * D], w[2 * D:]

    # tiny host matvecs (exact, fp32)
    u_full = lhs @ w_l + b  # (N, L)
    v_full = rhs @ w_r      # (N, R)

    f8 = ml_dtypes.float8_e4m3
    bf = ml_dtypes.bfloat16
    id_f8 = np.eye(P, dtype=f8)
    N = lhs.shape[0]

    def swizzle(a):
        # (N, X, Y) -> (N, P, NT, Y) with out[n, p, k] = a[n, k*128+p]
        return np.ascontiguousarray(
            a.reshape(N, NT, P, a.shape[2]).transpose(0, 2, 1, 3)
        )

    lhs_n = swizzle(lhs.astype(f8).astype(np.float32)).astype(f8)
    rhs_n = swizzle(rhs.astype(f8).astype(np.float32)).astype(f8)
    # d-major score operands; w_prod (and the fp8 range scale) fold into lhs^T
    lhs_tm = np.ascontiguousarray((lhs * (w_prod * W_SCALE)).transpose(0, 2, 1))
    rhs_tm = np.ascontiguousarray(rhs.transpose(0, 2, 1))
    lhs_t = swizzle(lhs_tm).astype(f8)
    rhs_t = swizzle(rhs_tm).astype(f8)

    in_maps = []
    for c in range(N_CORES):
        b0 = c * NB
        u_arr = np.ascontiguousarray(
            u_full[b0:b0 + NB].reshape(NB, NT, P).transpose(0, 2, 1)
        )  # (NB, 128, 8)
        # v is added on DVE in the PSUM (x64) domain, pre-descale
        v_bf = (v_full[b0:b0 + NB] * W_SCALE).astype(bf)  # (NB, R)
        vb_arr = np.ascontiguousarray(
            np.broadcast_to(v_bf[:, None, :], (NB, P, SEQ))
        )
        in_maps.append(
            {
                "lhs_n": lhs_n[b0:b0 + NB],
                "rhs_n": rhs_n[b0:b0 + NB],
                "lhs_t": lhs_t[b0:b0 + NB],
                "rhs_t": rhs_t[b0:b0 + NB],
                "u": u_arr,
                "vb": vb_arr,
                "id_f8": id_f8,
            }
        )
    return in_maps


def run_device(lhs, rhs, w, b, trace=False):
    """Returns (att_lhs, att_rhs, BassKernelResults)."""
    nc = _get_nc()
    in_maps = _prepare_in_maps(lhs, rhs, w, b)
    res = run_bass_kernel_spmd(
        nc, in_maps, core_ids=list(range(N_CORES)), trace=trace
    )
    N = lhs.shape[0]
    bf = ml_dtypes.bfloat16
    att_lhs = np.empty((N, SEQ, D), dtype=np.float32)
    att_rhs = np.empty((N, SEQ, D), dtype=np.float32)
    for c in range(N_CORES):
        b0 = c * NB
        att_lhs[b0:b0 + NB] = np.asarray(res.results[c]["att_lhs"]).view(bf).astype(np.float32)
        att_rhs[b0:b0 + NB] = np.asarray(res.results[c]["att_rhs"]).view(bf).astype(np.float32)
    return att_lhs, att_rhs, res


def kernel(lhs, rhs, w, b):
    import os

    lhs = np.asarray(lhs, dtype=np.float32)
    rhs = np.asarray(rhs, dtype=np.float32)
    assert lhs.shape == (N_CORES * NB, SEQ, D) and rhs.shape == lhs.shape, (
        f"expected ({N_CORES * NB}, {SEQ}, {D}) inputs, got {lhs.shape}/{rhs.shape}"
    )
    had = os.environ.get("BASS_NEVER_TRACE")
    os.environ["BASS_NEVER_TRACE"] = "1"
    try:
        att_lhs, att_rhs, _ = run_device(lhs, rhs, w, b, trace=False)
    finally:
        if had is None:
            os.environ.pop("BASS_NEVER_TRACE", None)
        else:
            os.environ["BASS_NEVER_TRACE"] = had
    lhs_out = np.concatenate([lhs, att_lhs], axis=2)
    rhs_out = np.concatenate([rhs, att_rhs], axis=2)
    return lhs_out, rhs_out
